# revision 8
# baseline (speedup 1.0000x reference)
"""Trainium2 Bass kernel for a 4-layer LSTM-style stack with local+global logits.

Computation (per example row x of the [16384, 512] input):
    h0 = 0, c0 = 0
    for i in 1..4:
        z  = [x, h_{i-1}] @ W{f,i,o,c} + b        (4 gates, K = 1024)
        c  = tanh(z_c) * sigmoid(z_i) + sigmoid(z_f) * c
        h  = sigmoid(z_o) * tanh(c)
        local_i = h @ Wl_i + bl_i
    global = [x, h4] @ Wg + bg
Returns (concat(local_1..4) [16384, 960], global [16384, 960]).

Strategy:
  - Data-parallel over 8 cores: 2048 rows each, weights replicated.
  - The input half of the concat never changes: Z = x @ W_top + b is computed
    once per example and reused by all 4 layers; layer 1 needs no matmul at
    all (h0 = 0, gates = act(Z)).
  - Activations are kept transposed in SBUF (features on partitions, examples
    on the free dim) so gate matmuls need no transposes: the host supplies
    x already transposed.  Logits are computed in natural layout (examples on
    partitions) using H/X tiles as the stationary operand, so outputs DMA out
    without any transpose either.
  - bf16 matmul operands (1 cycle/row on the PE); cell state kept fp32.
  - Each core processes its 2048 rows as 4 quarters of 512 examples,
    software-pipelined two-at-a-time to keep the PE busy across the
    sequential layer boundaries.
"""

import os
import sys

import numpy as np

for _p in ("/opt/trn_rl_repo", "/root/.axon_site/_ro/trn_rl_repo"):
    if os.path.isdir(_p) and _p not in sys.path:
        sys.path.insert(0, _p)

import ml_dtypes

import concourse.bass as bass
import concourse.tile as tile
from concourse import bacc, mybir
from concourse.bass_utils import run_bass_kernel_spmd

BF16 = mybir.dt.bfloat16
F32 = mybir.dt.float32
AF = mybir.ActivationFunctionType
ALU = mybir.AluOpType

N_CORES = 8
N = 16384
K = 512                  # input features
U = 512                  # hidden units
GF = 4 * U               # 2048 concatenated gate features (order f, i, o, c)
MC = N // N_CORES        # 2048 rows per core
NQ = 4                   # quarters per core
EXQ = MC // NQ           # 512 examples per quarter
ET = EXQ // 128          # 4 example tiles of 128 per quarter
NCLS = [64, 128, 256, 512]
OFFS = [0, 64, 192, 448]
TOT = 960
N_LAYERS = 4

LAST_RESULT = None       # BassKernelResults of the most recent run (for test.py)


def _build_program():
    """Build the SPMD Bass program (identical on every core)."""
    nc = bacc.Bacc("TRN2", target_bir_lowering=False, debug=False)

    xt_d = nc.dram_tensor("xt", [K, MC], BF16, kind="ExternalInput")
    wcat_d = nc.dram_tensor("wcat", [K + U, GF], BF16, kind="ExternalInput")
    wl_d = nc.dram_tensor("wl", [U, TOT], BF16, kind="ExternalInput")
    wg_d = nc.dram_tensor("wg", [K + U, TOT], BF16, kind="ExternalInput")
    bgate_d = nc.dram_tensor("bgate", [128, 16], F32, kind="ExternalInput")
    blrep_d = nc.dram_tensor("blrep", [128, TOT], F32, kind="ExternalInput")
    bgrep_d = nc.dram_tensor("bgrep", [128, TOT], F32, kind="ExternalInput")
    oloc_d = nc.dram_tensor("oloc", [MC, TOT], F32, kind="ExternalOutput")
    oglb_d = nc.dram_tensor("oglb", [MC, TOT], F32, kind="ExternalOutput")

    with tile.TileContext(nc) as tc:
        with (
            tc.tile_pool(name="wpool", bufs=1) as wpool,
            tc.tile_pool(name="xpool", bufs=3) as xpool,
            tc.tile_pool(name="zpool", bufs=2) as zpool,
            tc.tile_pool(name="hpool", bufs=3) as hpool,
            tc.tile_pool(name="cpool", bufs=2) as cpool,
            tc.tile_pool(name="gatep", bufs=4) as gatep,
            tc.tile_pool(name="prep", bufs=8) as prep,
            tc.tile_pool(name="ttp", bufs=2) as ttp,
            tc.tile_pool(name="tcp", bufs=4) as tcp,
            tc.tile_pool(name="lop", bufs=4) as lop,
            tc.tile_pool(name="glop", bufs=2) as glop,
            tc.tile_pool(name="gpsum", bufs=8, space="PSUM") as gpsum,
        ):
            # ---- resident weights/biases --------------------------------
            # DMA emission order matters: the first Z-phase matmul needs only
            # x(q0) + the first 512-column group of W_top, so those bytes go
            # first (W_top is split into [128, 512] column tiles to keep the
            # dependency granularity small).
            xs_pre = {}
            tiles = []
            wtop_sb = [[None] * 4 for _ in range(4)]   # [kt][column group]
            for kt in range(4):
                t = xpool.tile([128, EXQ], BF16, tag=f"x{kt}")
                nc.sync.dma_start(t[:], xt_d[kt * 128:(kt + 1) * 128, 0:EXQ])
                tiles.append(t)
                w = wpool.tile([128, 512], BF16, tag=f"wt{kt}g0")
                nc.sync.dma_start(
                    w[:], wcat_d[kt * 128:(kt + 1) * 128, 0:512])
                wtop_sb[kt][0] = w
            xs_pre[0] = tiles
            bgate_sb = wpool.tile([128, 16], F32, tag="bgate")
            nc.sync.dma_start(bgate_sb[:], bgate_d[:])
            for g in range(1, 4):
                for kt in range(4):
                    t = wpool.tile([128, 512], BF16, tag=f"wt{kt}g{g}")
                    nc.sync.dma_start(
                        t[:], wcat_d[kt * 128:(kt + 1) * 128,
                                     g * 512:(g + 1) * 512])
                    wtop_sb[kt][g] = t
            tiles = []
            for kt in range(4):
                t = xpool.tile([128, EXQ], BF16, tag=f"x{kt}")
                nc.sync.dma_start(
                    t[:], xt_d[kt * 128:(kt + 1) * 128, EXQ:2 * EXQ])
                tiles.append(t)
            xs_pre[1] = tiles
            wcat_sb = [None] * 8
            for kt in range(4, 8):
                t = wpool.tile([128, GF], BF16, tag=f"wcat{kt}")
                nc.sync.dma_start(t[:], wcat_d[kt * 128:(kt + 1) * 128, :])
                wcat_sb[kt] = t
            wl_sb = []
            for kt in range(4):
                t = wpool.tile([128, TOT], BF16, tag=f"wl{kt}")
                nc.sync.dma_start(t[:], wl_d[kt * 128:(kt + 1) * 128, :])
                wl_sb.append(t)
            wg_sb = []
            for kt in range(8):
                t = wpool.tile([128, TOT], BF16, tag=f"wg{kt}")
                nc.sync.dma_start(t[:], wg_d[kt * 128:(kt + 1) * 128, :])
                wg_sb.append(t)
            blrep_sb = wpool.tile([128, TOT], F32, tag="blrep")
            nc.sync.dma_start(blrep_sb[:], blrep_d[:])
            bgrep_sb = wpool.tile([128, TOT], F32, tag="bgrep")
            nc.sync.dma_start(bgrep_sb[:], bgrep_d[:])

            # per-quarter live state
            xs = [None] * NQ      # 4 X^T tiles [128, EXQ] bf16
            zs = [None] * NQ      # 16 Z tiles [128, EXQ] bf16 (bias folded in)
            hs = [None] * NQ      # 4 H^T tiles [128, EXQ] bf16 (current layer)
            cs = [None] * NQ      # 4 C tiles [128, EXQ] f32

            def stage_z(q):
                """DMA this quarter's x, compute Z = x @ W_top + b (bf16)."""
                if q in xs_pre:
                    xs[q] = xs_pre.pop(q)
                else:
                    xs[q] = []
                    for kt in range(4):
                        t = xpool.tile([128, EXQ], BF16, tag=f"x{kt}")
                        nc.sync.dma_start(
                            t[:], xt_d[kt * 128:(kt + 1) * 128,
                                       q * EXQ:(q + 1) * EXQ])
                        xs[q].append(t)
                zs[q] = []
                for of in range(16):
                    g, c = of // 4, of % 4
                    ps = gpsum.tile([128, EXQ], F32, tag="gp")
                    for kt in range(4):
                        nc.tensor.matmul(
                            ps[:], wtop_sb[kt][g][:, c * 128:(c + 1) * 128],
                            xs[q][kt][:], start=(kt == 0), stop=(kt == 3))
                    zt = zpool.tile([128, EXQ], BF16, tag=f"z{of}")
                    nc.scalar.activation(
                        zt[:], ps[:], AF.Identity,
                        bias=bgate_sb[:, of:of + 1])
                    zs[q].append(zt)

            def cand_update(q, t, ig, og, ch, fg):
                """c = ch*ig (+ fg*c); h = og * tanh(c).  fg None on layer 1."""
                if fg is None:
                    cn = cpool.tile([128, EXQ], BF16, tag=f"c{t}")
                    nc.vector.tensor_mul(cn[:], ig[:], ch[:])
                else:
                    t1 = ttp.tile([128, EXQ], BF16, tag="t1")
                    nc.vector.tensor_mul(t1[:], ig[:], ch[:])
                    t2 = ttp.tile([128, EXQ], BF16, tag="t2")
                    nc.vector.tensor_mul(t2[:], fg[:], cs[q][t][:])
                    cn = cpool.tile([128, EXQ], BF16, tag=f"c{t}")
                    nc.vector.tensor_add(cn[:], t1[:], t2[:])
                cs[q][t] = cn
                tc_t = tcp.tile([128, EXQ], BF16, tag="tc")
                nc.scalar.activation(tc_t[:], cn[:], AF.Tanh)
                hn = hpool.tile([128, EXQ], BF16, tag=f"h{t}")
                nc.vector.tensor_mul(hn[:], og[:], tc_t[:])
                hs[q][t] = hn

            def stage_l1(q):
                """Layer 1: h0 = 0 so gates come straight from Z (no matmul)."""
                hs[q] = [None] * 4
                cs[q] = [None] * 4
                for t in range(4):
                    ig = gatep.tile([128, EXQ], BF16, tag="g1")
                    nc.scalar.activation(ig[:], zs[q][4 + t][:], AF.Sigmoid)
                    og = gatep.tile([128, EXQ], BF16, tag="g2")
                    nc.scalar.activation(og[:], zs[q][8 + t][:], AF.Sigmoid)
                    ch = gatep.tile([128, EXQ], BF16, tag="g3")
                    nc.scalar.activation(ch[:], zs[q][12 + t][:], AF.Tanh)
                    cand_update(q, t, ig, og, ch, None)

            def emit_locals(q, layer, h_tiles):
                """local_i = h_i @ Wl_i + bl_i, natural layout, DMA out."""
                off, ncl = OFFS[layer], NCLS[layer]
                for e in range(ET):
                    ps = gpsum.tile([128, 512], F32, tag="gp")
                    for kt in range(4):
                        nc.tensor.matmul(
                            ps[:, 0:ncl],
                            h_tiles[kt][:, e * 128:(e + 1) * 128],
                            wl_sb[kt][:, off:off + ncl],
                            start=(kt == 0), stop=(kt == 3))
                    ot = lop.tile([128, 512], F32, tag="lo")
                    nc.vector.tensor_add(
                        ot[:, 0:ncl], ps[:, 0:ncl], blrep_sb[:, off:off + ncl])
                    r0 = q * EXQ + e * 128
                    nc.sync.dma_start(
                        oloc_d[r0:r0 + 128, off:off + ncl], ot[:, 0:ncl])

            def stage_layer(q, layer):
                """Layers 2..4: gates = act(Z + h @ W_bot); then locals of the
                previous layer (ready at the same time, keeps the PE busy)."""
                h_prev = hs[q]
                hs[q] = [None] * 4
                emit_locals(q, layer - 2, h_prev)
                for t in range(4):
                    gts = []
                    for g in range(4):  # f, i, o, c
                        of = g * 4 + t
                        ps = gpsum.tile([128, EXQ], F32, tag="gp")
                        for kt in range(4):
                            nc.tensor.matmul(
                                ps[:],
                                wcat_sb[4 + kt][:, of * 128:(of + 1) * 128],
                                h_prev[kt][:], start=(kt == 0), stop=(kt == 3))
                        pre = prep.tile([128, EXQ], BF16, tag="pre")
                        nc.vector.tensor_tensor(
                            pre[:], ps[:], zs[q][of][:], ALU.add)
                        gt = gatep.tile([128, EXQ], BF16, tag=f"g{g}")
                        nc.scalar.activation(
                            gt[:], pre[:], AF.Tanh if g == 3 else AF.Sigmoid)
                        gts.append(gt)
                    cand_update(q, t, gts[1], gts[2], gts[3], gts[0])

            def stage_gl(q):
                """locals of layer 4, then global = [x, h4] @ Wg + bg."""
                emit_locals(q, 3, hs[q])
                xh = xs[q] + hs[q]
                for e in range(ET):
                    gt = glop.tile([128, TOT], F32, tag="glo")
                    for s0, s1 in ((0, 512), (512, TOT)):
                        ps = gpsum.tile([128, 512], F32, tag="gp")
                        w = s1 - s0
                        for kt in range(8):
                            nc.tensor.matmul(
                                ps[:, 0:w],
                                xh[kt][:, e * 128:(e + 1) * 128],
                                wg_sb[kt][:, s0:s1],
                                start=(kt == 0), stop=(kt == 7))
                        nc.vector.tensor_add(
                            gt[:, s0:s1], ps[:, 0:w], bgrep_sb[:, s0:s1])
                    r0 = q * EXQ + e * 128
                    nc.sync.dma_start(oglb_d[r0:r0 + 128, :], gt[:])

            # ---- software-pipelined emission (2 quarters in flight) -----
            plan = [
                (0, "Z"), (1, "Z"), (0, "L1"), (1, "L1"),
                (0, 2), (1, 2), (0, 3), (1, 3), (0, 4), (1, 4),
                (0, "GL"), (2, "Z"), (2, "L1"), (1, "GL"),
                (3, "Z"), (3, "L1"),
                (2, 2), (3, 2), (2, 3), (3, 3), (2, 4), (3, 4),
                (2, "GL"), (3, "GL"),
            ]
            for q, s in plan:
                if s == "Z":
                    stage_z(q)
                elif s == "L1":
                    stage_l1(q)
                elif s == "GL":
                    stage_gl(q)
                else:
                    stage_layer(q, s)

    nc.compile()
    return nc


_PROGRAM = None


def _get_program():
    global _PROGRAM
    if _PROGRAM is None:
        _PROGRAM = _build_program()
    return _PROGRAM


def kernel(inputs, Wf, bf, Wi, bi, Wo, bo, Wc, bc,
           Wl0, bl0, Wl1, bl1, Wl2, bl2, Wl3, bl3, Wg, bg):
    global LAST_RESULT
    bf16 = ml_dtypes.bfloat16

    inputs = np.ascontiguousarray(np.asarray(inputs, dtype=np.float32))
    xt_all = inputs.T.astype(bf16)                    # [512, 16384]
    wcat = np.concatenate(
        [np.asarray(w, np.float32) for w in (Wf, Wi, Wo, Wc)],
        axis=1).astype(bf16)                          # [1024, 2048]
    bcat = np.concatenate(
        [np.asarray(b, np.float32) for b in (bf, bi, bo, bc)])  # [2048]
    bgate = np.ascontiguousarray(bcat.reshape(16, 128).T)       # [128, 16]
    wl = np.concatenate(
        [np.asarray(w, np.float32) for w in (Wl0, Wl1, Wl2, Wl3)],
        axis=1).astype(bf16)                          # [512, 960]
    blrep = np.ascontiguousarray(np.broadcast_to(
        np.concatenate([np.asarray(b, np.float32)
                        for b in (bl0, bl1, bl2, bl3)]), (128, TOT)))
    wg = np.asarray(Wg, np.float32).astype(bf16)      # [1024, 960]
    bgrep = np.ascontiguousarray(
        np.broadcast_to(np.asarray(bg, np.float32), (128, TOT)))

    in_maps = []
    for c in range(N_CORES):
        in_maps.append({
            "xt": np.ascontiguousarray(xt_all[:, c * MC:(c + 1) * MC]),
            "wcat": wcat, "wl": wl, "wg": wg,
            "bgate": bgate, "blrep": blrep, "bgrep": bgrep,
        })

    nc = _get_program()
    trace = os.environ.get("BASS_KERNEL_TRACE", "0") == "1"
    tmpdir = os.environ.get("BASS_KERNEL_TMPDIR") or None
    res = run_bass_kernel_spmd(
        nc, in_maps, list(range(N_CORES)), trace=trace, tmpdir=tmpdir)
    LAST_RESULT = res

    loc = np.concatenate([r["oloc"] for r in res.results], axis=0)
    glb = np.concatenate([r["oglb"] for r in res.results], axis=0)
    return loc, glb


# revision 9
# speedup vs baseline: 1.0222x; 1.0222x over previous
"""Trainium2 Bass kernel for a 4-layer LSTM-style stack with local+global logits.

Computation (per example row x of the [16384, 512] input):
    h0 = 0, c0 = 0
    for i in 1..4:
        z  = [x, h_{i-1}] @ W{f,i,o,c} + b        (4 gates, K = 1024)
        c  = tanh(z_c) * sigmoid(z_i) + sigmoid(z_f) * c
        h  = sigmoid(z_o) * tanh(c)
        local_i = h @ Wl_i + bl_i
    global = [x, h4] @ Wg + bg
Returns (concat(local_1..4) [16384, 960], global [16384, 960]).

Strategy:
  - Data-parallel over 8 cores: 2048 rows each, weights replicated.
  - The input half of the concat never changes: Z = x @ W_top + b is computed
    once per example and reused by all 4 layers; layer 1 needs no matmul at
    all (h0 = 0, gates = act(Z)).
  - Activations are kept transposed in SBUF (features on partitions, examples
    on the free dim) so gate matmuls need no transposes: the host supplies
    x already transposed.  Logits are computed in natural layout (examples on
    partitions) using H/X tiles as the stationary operand, so outputs DMA out
    without any transpose either.
  - bf16 matmul operands (1 cycle/row on the PE); cell state kept fp32.
  - Each core processes its 2048 rows as 4 quarters of 512 examples,
    software-pipelined two-at-a-time to keep the PE busy across the
    sequential layer boundaries.
"""

import os
import sys

import numpy as np

for _p in ("/opt/trn_rl_repo", "/root/.axon_site/_ro/trn_rl_repo"):
    if os.path.isdir(_p) and _p not in sys.path:
        sys.path.insert(0, _p)

import ml_dtypes

import concourse.bass as bass
import concourse.tile as tile
from concourse import bacc, mybir
from concourse.bass_utils import run_bass_kernel_spmd

BF16 = mybir.dt.bfloat16
F32 = mybir.dt.float32
AF = mybir.ActivationFunctionType
ALU = mybir.AluOpType

N_CORES = 8
N = 16384
K = 512                  # input features
U = 512                  # hidden units
GF = 4 * U               # 2048 concatenated gate features (order f, i, o, c)
MC = N // N_CORES        # 2048 rows per core
NQ = 4                   # quarters per core
EXQ = MC // NQ           # 512 examples per quarter
ET = EXQ // 128          # 4 example tiles of 128 per quarter
NCLS = [64, 128, 256, 512]
OFFS = [0, 64, 192, 448]
TOT = 960
N_LAYERS = 4

LAST_RESULT = None       # BassKernelResults of the most recent run (for test.py)


def _build_program():
    """Build the SPMD Bass program (identical on every core)."""
    nc = bacc.Bacc("TRN2", target_bir_lowering=False, debug=False)

    xt_d = nc.dram_tensor("xt", [K, MC], BF16, kind="ExternalInput")
    wcat_d = nc.dram_tensor("wcat", [K + U, GF], BF16, kind="ExternalInput")
    wl_d = nc.dram_tensor("wl", [U, TOT], BF16, kind="ExternalInput")
    wg_d = nc.dram_tensor("wg", [K + U, TOT], BF16, kind="ExternalInput")
    bgate_d = nc.dram_tensor("bgate", [128, 16], F32, kind="ExternalInput")
    blrep_d = nc.dram_tensor("blrep", [128, TOT], F32, kind="ExternalInput")
    bgrep_d = nc.dram_tensor("bgrep", [128, TOT], F32, kind="ExternalInput")
    oloc_d = nc.dram_tensor("oloc", [MC, TOT], F32, kind="ExternalOutput")
    oglb_d = nc.dram_tensor("oglb", [MC, TOT], F32, kind="ExternalOutput")

    with tile.TileContext(nc) as tc:
        with (
            tc.tile_pool(name="wpool", bufs=1) as wpool,
            tc.tile_pool(name="xpool", bufs=3) as xpool,
            tc.tile_pool(name="zpool", bufs=2) as zpool,
            tc.tile_pool(name="hpool", bufs=3) as hpool,
            tc.tile_pool(name="cpool", bufs=2) as cpool,
            tc.tile_pool(name="gatep", bufs=4) as gatep,
            tc.tile_pool(name="prep", bufs=8) as prep,
            tc.tile_pool(name="ttp", bufs=2) as ttp,
            tc.tile_pool(name="tcp", bufs=4) as tcp,
            tc.tile_pool(name="lop", bufs=4) as lop,
            tc.tile_pool(name="glop", bufs=2) as glop,
            tc.tile_pool(name="gpsum", bufs=8, space="PSUM") as gpsum,
        ):
            # ---- resident weights/biases --------------------------------
            # DMA emission order matters: the first Z-phase matmul needs only
            # x(q0) + the first 512-column group of W_top, so those bytes go
            # first (W_top is split into [128, 512] column tiles to keep the
            # dependency granularity small).
            xs_pre = {}
            tiles = []
            wtop_sb = [[None] * 4 for _ in range(4)]   # [kt][column group]
            for kt in range(4):
                t = xpool.tile([128, EXQ], BF16, tag=f"x{kt}")
                nc.sync.dma_start(t[:], xt_d[kt * 128:(kt + 1) * 128, 0:EXQ])
                tiles.append(t)
                w = wpool.tile([128, 512], BF16, tag=f"wt{kt}g0")
                nc.sync.dma_start(
                    w[:], wcat_d[kt * 128:(kt + 1) * 128, 0:512])
                wtop_sb[kt][0] = w
            xs_pre[0] = tiles
            bgate_sb = wpool.tile([128, 16], F32, tag="bgate")
            nc.sync.dma_start(bgate_sb[:], bgate_d[:])
            for g in range(1, 4):
                for kt in range(4):
                    t = wpool.tile([128, 512], BF16, tag=f"wt{kt}g{g}")
                    nc.sync.dma_start(
                        t[:], wcat_d[kt * 128:(kt + 1) * 128,
                                     g * 512:(g + 1) * 512])
                    wtop_sb[kt][g] = t
            tiles = []
            for kt in range(4):
                t = xpool.tile([128, EXQ], BF16, tag=f"x{kt}")
                nc.sync.dma_start(
                    t[:], xt_d[kt * 128:(kt + 1) * 128, EXQ:2 * EXQ])
                tiles.append(t)
            xs_pre[1] = tiles
            wcat_sb = [None] * 8
            for kt in range(4, 8):
                t = wpool.tile([128, GF], BF16, tag=f"wcat{kt}")
                nc.sync.dma_start(t[:], wcat_d[kt * 128:(kt + 1) * 128, :])
                wcat_sb[kt] = t
            wl_sb = []
            for kt in range(4):
                t = wpool.tile([128, TOT], BF16, tag=f"wl{kt}")
                nc.sync.dma_start(t[:], wl_d[kt * 128:(kt + 1) * 128, :])
                wl_sb.append(t)
            wg_sb = []
            for kt in range(8):
                t = wpool.tile([128, TOT], BF16, tag=f"wg{kt}")
                nc.sync.dma_start(t[:], wg_d[kt * 128:(kt + 1) * 128, :])
                wg_sb.append(t)
            blrep_sb = wpool.tile([128, TOT], F32, tag="blrep")
            nc.sync.dma_start(blrep_sb[:], blrep_d[:])
            bgrep_sb = wpool.tile([128, TOT], F32, tag="bgrep")
            nc.sync.dma_start(bgrep_sb[:], bgrep_d[:])

            # per-quarter live state
            xs = [None] * NQ      # 4 X^T tiles [128, EXQ] bf16
            zs = [None] * NQ      # 16 Z tiles [128, EXQ] bf16 (bias folded in)
            hs = [None] * NQ      # 4 H^T tiles [128, EXQ] bf16 (current layer)
            cs = [None] * NQ      # 4 C tiles [128, EXQ] f32

            def stage_z(q):
                """DMA this quarter's x, compute Z = x @ W_top + b (bf16)."""
                if q in xs_pre:
                    xs[q] = xs_pre.pop(q)
                else:
                    xs[q] = []
                    for kt in range(4):
                        t = xpool.tile([128, EXQ], BF16, tag=f"x{kt}")
                        nc.sync.dma_start(
                            t[:], xt_d[kt * 128:(kt + 1) * 128,
                                       q * EXQ:(q + 1) * EXQ])
                        xs[q].append(t)
                zs[q] = []
                for of in range(16):
                    g, c = of // 4, of % 4
                    ps = gpsum.tile([128, EXQ], F32, tag="gp")
                    for kt in range(4):
                        nc.tensor.matmul(
                            ps[:], wtop_sb[kt][g][:, c * 128:(c + 1) * 128],
                            xs[q][kt][:], start=(kt == 0), stop=(kt == 3))
                    zt = zpool.tile([128, EXQ], BF16, tag=f"z{of}")
                    nc.vector.tensor_scalar(
                        zt[:], ps[:], bgate_sb[:, of:of + 1], None, ALU.add)
                    zs[q].append(zt)

            def cand_update(q, t, ig, og, ch, fg):
                """c = ch*ig (+ fg*c); h = og * tanh(c).  fg None on layer 1."""
                if fg is None:
                    cn = cpool.tile([128, EXQ], BF16, tag=f"c{t}")
                    nc.vector.tensor_mul(cn[:], ig[:], ch[:])
                else:
                    t1 = ttp.tile([128, EXQ], BF16, tag="t1")
                    nc.vector.tensor_mul(t1[:], ig[:], ch[:])
                    t2 = ttp.tile([128, EXQ], BF16, tag="t2")
                    nc.vector.tensor_mul(t2[:], fg[:], cs[q][t][:])
                    cn = cpool.tile([128, EXQ], BF16, tag=f"c{t}")
                    nc.vector.tensor_add(cn[:], t1[:], t2[:])
                cs[q][t] = cn
                tc_t = tcp.tile([128, EXQ], BF16, tag="tc")
                nc.scalar.activation(tc_t[:], cn[:], AF.Tanh)
                hn = hpool.tile([128, EXQ], BF16, tag=f"h{t}")
                nc.vector.tensor_mul(hn[:], og[:], tc_t[:])
                hs[q][t] = hn

            def stage_l1(q):
                """Layer 1: h0 = 0 so gates come straight from Z (no matmul)."""
                hs[q] = [None] * 4
                cs[q] = [None] * 4
                for t in range(4):
                    ig = gatep.tile([128, EXQ], BF16, tag="g1")
                    nc.scalar.activation(ig[:], zs[q][4 + t][:], AF.Sigmoid)
                    og = gatep.tile([128, EXQ], BF16, tag="g2")
                    nc.scalar.activation(og[:], zs[q][8 + t][:], AF.Sigmoid)
                    ch = gatep.tile([128, EXQ], BF16, tag="g3")
                    nc.scalar.activation(ch[:], zs[q][12 + t][:], AF.Tanh)
                    cand_update(q, t, ig, og, ch, None)

            def emit_locals(q, layer, h_tiles):
                """local_i = h_i @ Wl_i + bl_i, natural layout, DMA out."""
                off, ncl = OFFS[layer], NCLS[layer]
                for e in range(ET):
                    ps = gpsum.tile([128, 512], F32, tag="gp")
                    for kt in range(4):
                        nc.tensor.matmul(
                            ps[:, 0:ncl],
                            h_tiles[kt][:, e * 128:(e + 1) * 128],
                            wl_sb[kt][:, off:off + ncl],
                            start=(kt == 0), stop=(kt == 3))
                    ot = lop.tile([128, 512], F32, tag="lo")
                    nc.vector.tensor_add(
                        ot[:, 0:ncl], ps[:, 0:ncl], blrep_sb[:, off:off + ncl])
                    r0 = q * EXQ + e * 128
                    nc.sync.dma_start(
                        oloc_d[r0:r0 + 128, off:off + ncl], ot[:, 0:ncl])

            def stage_layer(q, layer):
                """Layers 2..4: gates = act(Z + h @ W_bot); then locals of the
                previous layer (ready at the same time, keeps the PE busy)."""
                h_prev = hs[q]
                hs[q] = [None] * 4
                emit_locals(q, layer - 2, h_prev)
                for t in range(4):
                    gts = []
                    for g in range(4):  # f, i, o, c
                        of = g * 4 + t
                        ps = gpsum.tile([128, EXQ], F32, tag="gp")
                        for kt in range(4):
                            nc.tensor.matmul(
                                ps[:],
                                wcat_sb[4 + kt][:, of * 128:(of + 1) * 128],
                                h_prev[kt][:], start=(kt == 0), stop=(kt == 3))
                        pre = prep.tile([128, EXQ], BF16, tag="pre")
                        nc.vector.tensor_tensor(
                            pre[:], ps[:], zs[q][of][:], ALU.add)
                        gt = gatep.tile([128, EXQ], BF16, tag=f"g{g}")
                        nc.scalar.activation(
                            gt[:], pre[:], AF.Tanh if g == 3 else AF.Sigmoid)
                        gts.append(gt)
                    cand_update(q, t, gts[1], gts[2], gts[3], gts[0])

            def stage_gl(q):
                """locals of layer 4, then global = [x, h4] @ Wg + bg."""
                emit_locals(q, 3, hs[q])
                xh = xs[q] + hs[q]
                for e in range(ET):
                    gt = glop.tile([128, TOT], F32, tag="glo")
                    for s0, s1 in ((0, 512), (512, TOT)):
                        ps = gpsum.tile([128, 512], F32, tag="gp")
                        w = s1 - s0
                        for kt in range(8):
                            nc.tensor.matmul(
                                ps[:, 0:w],
                                xh[kt][:, e * 128:(e + 1) * 128],
                                wg_sb[kt][:, s0:s1],
                                start=(kt == 0), stop=(kt == 7))
                        nc.vector.tensor_add(
                            gt[:, s0:s1], ps[:, 0:w], bgrep_sb[:, s0:s1])
                    r0 = q * EXQ + e * 128
                    nc.sync.dma_start(oglb_d[r0:r0 + 128, :], gt[:])

            # ---- software-pipelined emission (2 quarters in flight) -----
            plan = [
                (0, "Z"), (1, "Z"), (0, "L1"), (1, "L1"),
                (0, 2), (1, 2), (0, 3), (1, 3), (0, 4), (1, 4),
                (0, "GL"), (2, "Z"), (2, "L1"), (1, "GL"),
                (3, "Z"), (3, "L1"),
                (2, 2), (3, 2), (2, 3), (3, 3), (2, 4), (3, 4),
                (2, "GL"), (3, "GL"),
            ]
            for q, s in plan:
                if s == "Z":
                    stage_z(q)
                elif s == "L1":
                    stage_l1(q)
                elif s == "GL":
                    stage_gl(q)
                else:
                    stage_layer(q, s)

    nc.compile()
    return nc


_PROGRAM = None


def _get_program():
    global _PROGRAM
    if _PROGRAM is None:
        _PROGRAM = _build_program()
    return _PROGRAM


def kernel(inputs, Wf, bf, Wi, bi, Wo, bo, Wc, bc,
           Wl0, bl0, Wl1, bl1, Wl2, bl2, Wl3, bl3, Wg, bg):
    global LAST_RESULT
    bf16 = ml_dtypes.bfloat16

    inputs = np.ascontiguousarray(np.asarray(inputs, dtype=np.float32))
    xt_all = inputs.T.astype(bf16)                    # [512, 16384]
    wcat = np.concatenate(
        [np.asarray(w, np.float32) for w in (Wf, Wi, Wo, Wc)],
        axis=1).astype(bf16)                          # [1024, 2048]
    bcat = np.concatenate(
        [np.asarray(b, np.float32) for b in (bf, bi, bo, bc)])  # [2048]
    bgate = np.ascontiguousarray(bcat.reshape(16, 128).T)       # [128, 16]
    wl = np.concatenate(
        [np.asarray(w, np.float32) for w in (Wl0, Wl1, Wl2, Wl3)],
        axis=1).astype(bf16)                          # [512, 960]
    blrep = np.ascontiguousarray(np.broadcast_to(
        np.concatenate([np.asarray(b, np.float32)
                        for b in (bl0, bl1, bl2, bl3)]), (128, TOT)))
    wg = np.asarray(Wg, np.float32).astype(bf16)      # [1024, 960]
    bgrep = np.ascontiguousarray(
        np.broadcast_to(np.asarray(bg, np.float32), (128, TOT)))

    in_maps = []
    for c in range(N_CORES):
        in_maps.append({
            "xt": np.ascontiguousarray(xt_all[:, c * MC:(c + 1) * MC]),
            "wcat": wcat, "wl": wl, "wg": wg,
            "bgate": bgate, "blrep": blrep, "bgrep": bgrep,
        })

    nc = _get_program()
    trace = os.environ.get("BASS_KERNEL_TRACE", "0") == "1"
    tmpdir = os.environ.get("BASS_KERNEL_TMPDIR") or None
    res = run_bass_kernel_spmd(
        nc, in_maps, list(range(N_CORES)), trace=trace, tmpdir=tmpdir)
    LAST_RESULT = res

    loc = np.concatenate([r["oloc"] for r in res.results], axis=0)
    glb = np.concatenate([r["oglb"] for r in res.results], axis=0)
    return loc, glb


# revision 10
# speedup vs baseline: 1.0278x; 1.0056x over previous
"""Trainium2 Bass kernel for a 4-layer LSTM-style stack with local+global logits.

Computation (per example row x of the [16384, 512] input):
    h0 = 0, c0 = 0
    for i in 1..4:
        z  = [x, h_{i-1}] @ W{f,i,o,c} + b        (4 gates, K = 1024)
        c  = tanh(z_c) * sigmoid(z_i) + sigmoid(z_f) * c
        h  = sigmoid(z_o) * tanh(c)
        local_i = h @ Wl_i + bl_i
    global = [x, h4] @ Wg + bg
Returns (concat(local_1..4) [16384, 960], global [16384, 960]).

Strategy:
  - Data-parallel over 8 cores: 2048 rows each, weights replicated.
  - The input half of the concat never changes: Z = x @ W_top + b is computed
    once per example and reused by all 4 layers; layer 1 needs no matmul at
    all (h0 = 0, gates = act(Z)).
  - Activations are kept transposed in SBUF (features on partitions, examples
    on the free dim) so gate matmuls need no transposes: the host supplies
    x already transposed.  Logits are computed in natural layout (examples on
    partitions) using H/X tiles as the stationary operand, so outputs DMA out
    without any transpose either.
  - bf16 matmul operands (1 cycle/row on the PE); cell state kept fp32.
  - Each core processes its 2048 rows as 4 quarters of 512 examples,
    software-pipelined two-at-a-time to keep the PE busy across the
    sequential layer boundaries.
"""

import os
import sys

import numpy as np

for _p in ("/opt/trn_rl_repo", "/root/.axon_site/_ro/trn_rl_repo"):
    if os.path.isdir(_p) and _p not in sys.path:
        sys.path.insert(0, _p)

import ml_dtypes

import concourse.bass as bass
import concourse.tile as tile
from concourse import bacc, mybir
from concourse.bass_utils import run_bass_kernel_spmd

BF16 = mybir.dt.bfloat16
F32 = mybir.dt.float32
AF = mybir.ActivationFunctionType
ALU = mybir.AluOpType

N_CORES = 8
N = 16384
K = 512                  # input features
U = 512                  # hidden units
GF = 4 * U               # 2048 concatenated gate features (order f, i, o, c)
MC = N // N_CORES        # 2048 rows per core
NQ = 4                   # quarters per core
EXQ = MC // NQ           # 512 examples per quarter
ET = EXQ // 128          # 4 example tiles of 128 per quarter
NCLS = [64, 128, 256, 512]
OFFS = [0, 64, 192, 448]
TOT = 960
N_LAYERS = 4

LAST_RESULT = None       # BassKernelResults of the most recent run (for test.py)


def _build_program():
    """Build the SPMD Bass program (identical on every core)."""
    nc = bacc.Bacc("TRN2", target_bir_lowering=False, debug=False)

    xt_d = nc.dram_tensor("xt", [K, MC], BF16, kind="ExternalInput")
    wcat_d = nc.dram_tensor("wcat", [K + U, GF], BF16, kind="ExternalInput")
    wl_d = nc.dram_tensor("wl", [U, TOT], BF16, kind="ExternalInput")
    wg_d = nc.dram_tensor("wg", [K + U, TOT], BF16, kind="ExternalInput")
    bgate_d = nc.dram_tensor("bgate", [128, 16], F32, kind="ExternalInput")
    blrep_d = nc.dram_tensor("blrep", [128, TOT], F32, kind="ExternalInput")
    bgrep_d = nc.dram_tensor("bgrep", [128, TOT], F32, kind="ExternalInput")
    oloc_d = nc.dram_tensor("oloc", [MC, TOT], F32, kind="ExternalOutput")
    oglb_d = nc.dram_tensor("oglb", [MC, TOT], F32, kind="ExternalOutput")

    with tile.TileContext(nc) as tc:
        with (
            tc.tile_pool(name="wpool", bufs=1) as wpool,
            tc.tile_pool(name="xpool", bufs=3) as xpool,
            tc.tile_pool(name="zpool", bufs=2) as zpool,
            tc.tile_pool(name="hpool", bufs=3) as hpool,
            tc.tile_pool(name="cpool", bufs=2) as cpool,
            tc.tile_pool(name="gatep", bufs=3) as gatep,
            tc.tile_pool(name="prep", bufs=6) as prep,
            tc.tile_pool(name="ttp", bufs=2) as ttp,
            tc.tile_pool(name="tcp", bufs=3) as tcp,
            tc.tile_pool(name="lop", bufs=4) as lop,
            tc.tile_pool(name="glop", bufs=2) as glop,
            tc.tile_pool(name="gpsum", bufs=8, space="PSUM") as gpsum,
        ):
            # ---- resident weights/biases --------------------------------
            # DMA emission order matters: the first Z-phase matmul needs only
            # x(q0) + the first 512-column group of W_top, so those bytes go
            # first (W_top is split into [128, 512] column tiles to keep the
            # dependency granularity small).
            xs_pre = {}
            tiles = []
            wtop_sb = [[None] * 4 for _ in range(4)]   # [kt][column group]
            for kt in range(4):
                t = xpool.tile([128, EXQ], BF16, tag=f"x{kt}")
                nc.sync.dma_start(t[:], xt_d[kt * 128:(kt + 1) * 128, 0:EXQ])
                tiles.append(t)
                w = wpool.tile([128, 512], BF16, tag=f"wt{kt}g0")
                nc.sync.dma_start(
                    w[:], wcat_d[kt * 128:(kt + 1) * 128, 0:512])
                wtop_sb[kt][0] = w
            xs_pre[0] = tiles
            bgate_sb = wpool.tile([128, 16], F32, tag="bgate")
            nc.sync.dma_start(bgate_sb[:], bgate_d[:])
            for g in range(1, 4):
                for kt in range(4):
                    t = wpool.tile([128, 512], BF16, tag=f"wt{kt}g{g}")
                    nc.sync.dma_start(
                        t[:], wcat_d[kt * 128:(kt + 1) * 128,
                                     g * 512:(g + 1) * 512])
                    wtop_sb[kt][g] = t
            tiles = []
            for kt in range(4):
                t = xpool.tile([128, EXQ], BF16, tag=f"x{kt}")
                nc.sync.dma_start(
                    t[:], xt_d[kt * 128:(kt + 1) * 128, EXQ:2 * EXQ])
                tiles.append(t)
            xs_pre[1] = tiles
            wcat_sb = [None] * 8
            for kt in range(4, 8):
                t = wpool.tile([128, GF], BF16, tag=f"wcat{kt}")
                nc.sync.dma_start(t[:], wcat_d[kt * 128:(kt + 1) * 128, :])
                wcat_sb[kt] = t
            wl_sb = []
            for kt in range(4):
                t = wpool.tile([128, TOT], BF16, tag=f"wl{kt}")
                nc.sync.dma_start(t[:], wl_d[kt * 128:(kt + 1) * 128, :])
                wl_sb.append(t)
            wg_sb = []
            for kt in range(8):
                t = wpool.tile([128, TOT], BF16, tag=f"wg{kt}")
                nc.sync.dma_start(t[:], wg_d[kt * 128:(kt + 1) * 128, :])
                wg_sb.append(t)
            blrep_sb = wpool.tile([128, TOT], F32, tag="blrep")
            nc.sync.dma_start(blrep_sb[:], blrep_d[:])
            bgrep_sb = wpool.tile([128, TOT], F32, tag="bgrep")
            nc.sync.dma_start(bgrep_sb[:], bgrep_d[:])

            # per-quarter live state
            xs = [None] * NQ      # 4 X^T tiles [128, EXQ] bf16
            zs = [None] * NQ      # 16 Z tiles [128, EXQ] bf16 (bias folded in)
            hs = [None] * NQ      # 4 H^T tiles [128, EXQ] bf16 (current layer)
            cs = [None] * NQ      # 4 C tiles [128, EXQ] f32

            def stage_z(q):
                """DMA this quarter's x, compute Z = x @ W_top + b (bf16)."""
                if q in xs_pre:
                    xs[q] = xs_pre.pop(q)
                else:
                    xs[q] = []
                    for kt in range(4):
                        t = xpool.tile([128, EXQ], BF16, tag=f"x{kt}")
                        nc.sync.dma_start(
                            t[:], xt_d[kt * 128:(kt + 1) * 128,
                                       q * EXQ:(q + 1) * EXQ])
                        xs[q].append(t)
                zs[q] = []
                for of in range(16):
                    g, c = of // 4, of % 4
                    ps = gpsum.tile([128, EXQ], F32, tag="gp")
                    for kt in range(4):
                        nc.tensor.matmul(
                            ps[:], wtop_sb[kt][g][:, c * 128:(c + 1) * 128],
                            xs[q][kt][:], start=(kt == 0), stop=(kt == 3))
                    zt = zpool.tile([128, EXQ], BF16, tag=f"z{of}")
                    nc.vector.tensor_scalar(
                        zt[:], ps[:], bgate_sb[:, of:of + 1], None, ALU.add)
                    zs[q].append(zt)

            def cand_update(q, t, ig, og, ch, fg):
                """c = ch*ig (+ fg*c); h = og * tanh(c).  fg None on layer 1."""
                if fg is None:
                    cn = cpool.tile([128, EXQ], BF16, tag=f"c{t}")
                    nc.vector.tensor_mul(cn[:], ig[:], ch[:])
                else:
                    t1 = ttp.tile([128, EXQ], BF16, tag="t1")
                    nc.vector.tensor_mul(t1[:], ig[:], ch[:])
                    t2 = ttp.tile([128, EXQ], BF16, tag="t2")
                    nc.vector.tensor_mul(t2[:], fg[:], cs[q][t][:])
                    cn = cpool.tile([128, EXQ], BF16, tag=f"c{t}")
                    nc.vector.tensor_add(cn[:], t1[:], t2[:])
                cs[q][t] = cn
                tc_t = tcp.tile([128, EXQ], BF16, tag="tc")
                nc.scalar.activation(tc_t[:], cn[:], AF.Tanh)
                hn = hpool.tile([128, EXQ], BF16, tag=f"h{t}")
                nc.vector.tensor_mul(hn[:], og[:], tc_t[:])
                hs[q][t] = hn

            def stage_l1(q):
                """Layer 1: h0 = 0 so gates come straight from Z (no matmul)."""
                hs[q] = [None] * 4
                cs[q] = [None] * 4
                for t in range(4):
                    ig = gatep.tile([128, EXQ], BF16, tag="g1")
                    nc.scalar.activation(ig[:], zs[q][4 + t][:], AF.Sigmoid)
                    og = gatep.tile([128, EXQ], BF16, tag="g2")
                    nc.scalar.activation(og[:], zs[q][8 + t][:], AF.Sigmoid)
                    ch = gatep.tile([128, EXQ], BF16, tag="g3")
                    nc.scalar.activation(ch[:], zs[q][12 + t][:], AF.Tanh)
                    cand_update(q, t, ig, og, ch, None)

            def emit_locals(q, layer, h_tiles):
                """local_i = h_i @ Wl_i + bl_i, natural layout, DMA out."""
                off, ncl = OFFS[layer], NCLS[layer]
                for e in range(ET):
                    ps = gpsum.tile([128, 512], F32, tag="gp")
                    for kt in range(4):
                        nc.tensor.matmul(
                            ps[:, 0:ncl],
                            h_tiles[kt][:, e * 128:(e + 1) * 128],
                            wl_sb[kt][:, off:off + ncl],
                            start=(kt == 0), stop=(kt == 3))
                    ot = lop.tile([128, 512], F32, tag="lo")
                    nc.vector.tensor_add(
                        ot[:, 0:ncl], ps[:, 0:ncl], blrep_sb[:, off:off + ncl])
                    r0 = q * EXQ + e * 128
                    nc.sync.dma_start(
                        oloc_d[r0:r0 + 128, off:off + ncl], ot[:, 0:ncl])

            def stage_layer(q, layer):
                """Layers 2..4: gates = act(Z + h @ W_bot); then locals of the
                previous layer (ready at the same time, keeps the PE busy)."""
                h_prev = hs[q]
                hs[q] = [None] * 4
                emit_locals(q, layer - 2, h_prev)
                for t in range(4):
                    gts = []
                    for g in range(4):  # f, i, o, c
                        of = g * 4 + t
                        ps = gpsum.tile([128, EXQ], F32, tag="gp")
                        for kt in range(4):
                            nc.tensor.matmul(
                                ps[:],
                                wcat_sb[4 + kt][:, of * 128:(of + 1) * 128],
                                h_prev[kt][:], start=(kt == 0), stop=(kt == 3))
                        pre = prep.tile([128, EXQ], BF16, tag="pre")
                        nc.vector.tensor_tensor(
                            pre[:], ps[:], zs[q][of][:], ALU.add)
                        gt = gatep.tile([128, EXQ], BF16, tag=f"g{g}")
                        nc.scalar.activation(
                            gt[:], pre[:], AF.Tanh if g == 3 else AF.Sigmoid)
                        gts.append(gt)
                    cand_update(q, t, gts[1], gts[2], gts[3], gts[0])

            def stage_gl(q):
                """locals of layer 4, then global = [x, h4] @ Wg + bg."""
                emit_locals(q, 3, hs[q])
                xh = xs[q] + hs[q]
                for e in range(ET):
                    gt = glop.tile([128, TOT], F32, tag="glo")
                    for s0, s1 in ((0, 512), (512, TOT)):
                        ps = gpsum.tile([128, 512], F32, tag="gp")
                        w = s1 - s0
                        for kt in range(8):
                            nc.tensor.matmul(
                                ps[:, 0:w],
                                xh[kt][:, e * 128:(e + 1) * 128],
                                wg_sb[kt][:, s0:s1],
                                start=(kt == 0), stop=(kt == 7))
                        nc.vector.tensor_add(
                            gt[:, s0:s1], ps[:, 0:w], bgrep_sb[:, s0:s1])
                    r0 = q * EXQ + e * 128
                    nc.sync.dma_start(oglb_d[r0:r0 + 128, :], gt[:])

            # ---- software-pipelined emission (2 quarters in flight) -----
            plan = [
                (0, "Z"), (1, "Z"), (0, "L1"), (1, "L1"),
                (0, 2), (1, 2), (0, 3), (1, 3), (0, 4), (1, 4),
                (0, "GL"), (2, "Z"), (2, "L1"), (1, "GL"),
                (3, "Z"), (3, "L1"),
                (2, 2), (3, 2), (2, 3), (3, 3), (2, 4), (3, 4),
                (2, "GL"), (3, "GL"),
            ]
            for q, s in plan:
                if s == "Z":
                    stage_z(q)
                elif s == "L1":
                    stage_l1(q)
                elif s == "GL":
                    stage_gl(q)
                else:
                    stage_layer(q, s)

    nc.compile()
    return nc


_PROGRAM = None


def _get_program():
    global _PROGRAM
    if _PROGRAM is None:
        _PROGRAM = _build_program()
    return _PROGRAM


def kernel(inputs, Wf, bf, Wi, bi, Wo, bo, Wc, bc,
           Wl0, bl0, Wl1, bl1, Wl2, bl2, Wl3, bl3, Wg, bg):
    global LAST_RESULT
    bf16 = ml_dtypes.bfloat16

    inputs = np.ascontiguousarray(np.asarray(inputs, dtype=np.float32))
    xt_all = inputs.T.astype(bf16)                    # [512, 16384]
    wcat = np.concatenate(
        [np.asarray(w, np.float32) for w in (Wf, Wi, Wo, Wc)],
        axis=1).astype(bf16)                          # [1024, 2048]
    bcat = np.concatenate(
        [np.asarray(b, np.float32) for b in (bf, bi, bo, bc)])  # [2048]
    bgate = np.ascontiguousarray(bcat.reshape(16, 128).T)       # [128, 16]
    wl = np.concatenate(
        [np.asarray(w, np.float32) for w in (Wl0, Wl1, Wl2, Wl3)],
        axis=1).astype(bf16)                          # [512, 960]
    blrep = np.ascontiguousarray(np.broadcast_to(
        np.concatenate([np.asarray(b, np.float32)
                        for b in (bl0, bl1, bl2, bl3)]), (128, TOT)))
    wg = np.asarray(Wg, np.float32).astype(bf16)      # [1024, 960]
    bgrep = np.ascontiguousarray(
        np.broadcast_to(np.asarray(bg, np.float32), (128, TOT)))

    in_maps = []
    for c in range(N_CORES):
        in_maps.append({
            "xt": np.ascontiguousarray(xt_all[:, c * MC:(c + 1) * MC]),
            "wcat": wcat, "wl": wl, "wg": wg,
            "bgate": bgate, "blrep": blrep, "bgrep": bgrep,
        })

    nc = _get_program()
    trace = os.environ.get("BASS_KERNEL_TRACE", "0") == "1"
    tmpdir = os.environ.get("BASS_KERNEL_TMPDIR") or None
    res = run_bass_kernel_spmd(
        nc, in_maps, list(range(N_CORES)), trace=trace, tmpdir=tmpdir)
    LAST_RESULT = res

    loc = np.concatenate([r["oloc"] for r in res.results], axis=0)
    glb = np.concatenate([r["oglb"] for r in res.results], axis=0)
    return loc, glb
